# revision 11
# baseline (speedup 1.0000x reference)
"""Trainium2 Bass kernel for nn_Former_Mobile (mobile-former style cross-attention).

Computation (per batch item n):
    kv   = relu6(global_feature @ W_kv^T + b_kv)        # [m=8, 2c]
    K, V = kv[:, :c], kv[:, c:]                         # [8, c=384]
    q    = x reshaped [hw=3136, c]
    attn = softmax(q @ K^T)                             # [hw, 8]
    out  = (attn @ V) reshaped back + x                 # [c, hw]

Sharding: data-parallel over batch n across 8 NeuronCores (4 items each);
W_kv/b_kv replicated (bias folded into an extra contraction row host-side).

All I/O and matmul operands are fp16 (halves HBM traffic vs fp32 and runs the
PE at full streaming rate; fp32/f32r streams at half rate). PSUM accumulation
stays fp32. Outputs are converted back to fp32 on the host.

Device pipeline per core:
  kv phase: kvT chunks [c_chunk, nm] = wt-slices @ gft (so K^T needs no
      on-device transpose), relu6 -> KT[kc] fp16. V for all items via one
      accumulated matmul -> V_all [nm=32, c], relu6.
  per item n (output phase software-pipelined one item behind):
    x loads: ONE merged DMA per item (3 c-chunks), alternating between the
             two HWDGE queues (sync/scalar) to overlap transfers and hide
             the per-DMA completion latency.
    V_rep  = rep_sel_n^T @ V_all: V_n replicated at partitions 0/32/64/96
             (zero elsewhere) so mm2 can run as k=32 row-group matmuls.
    scores [hw_p, m] directly: lhsT = x-tile [c128, hw<=128] (x is the
             stationary operand), rhs = KT[kc][:, n*8:+8], accumulated over
             3 c-chunks into one psum bank [128, 25*8].
    softmax along free dim m (128-way partition parallel), attn written fp16
             into attn_pad [128, 25*32] (m padded to 32, pads pre-zeroed).
    T2: 7 batched PE transposes of [128, 128] blocks (4 hw-tiles each);
             t-slabs land at partitions 32*(t%4) which are legal AP starts.
    mm2: out[c128, hw] = V_rep row-group k=32 matmul + identity-matmul
             accumulate of x (the residual, done on the otherwise-idle PE),
             then a plain psum->sbuf fp16 copy alternating DVE/ACT (a copy
             is the cheapest way off PSUM; splitting halves the critical
             path that bound v2). Merged store DMA per item, alternating
             gpsimd (SWDGE) / scalar (HWDGE) queues.
"""

import sys

if "/opt/trn_rl_repo" not in sys.path:
    sys.path.insert(0, "/opt/trn_rl_repo")

import numpy as np

N, C, H, W = 32, 384, 56, 56
HW = H * W                      # 3136
M, D = 8, 768
N_CORES = 8
N_LOC = N // N_CORES            # 4 batch items per core
NM = N_LOC * M                  # 32 kv rows per core
D1P = 896                       # 768 + bias row, zero-padded to 7*128
KC = C // 128                   # 3 contraction chunks over c
P = 128
NT = 25                         # hw tiles: 24 x 128 + 1 x 64
MP = 32                         # m padded to 32 for batched transposes
XPAD = 3584                     # per-chunk tile free size (3136 + slack for
                                # the strided residual rearrange views)

_cache = {}
last_results = None


def _build():
    from concourse import bacc, tile, mybir

    f16 = mybir.dt.float16
    f32 = mybir.dt.float32
    Alu = mybir.AluOpType
    Act = mybir.ActivationFunctionType
    PSUM = tile.bass.MemorySpace.PSUM

    nc = bacc.Bacc("TRN2", target_bir_lowering=False, debug=False,
                   num_devices=N_CORES)

    xs_d = nc.dram_tensor("xs", [N_LOC, C, HW], f16, kind="ExternalInput")
    gft_d = nc.dram_tensor("gft", [D1P, NM], f16, kind="ExternalInput")
    wt_d = nc.dram_tensor("wt", [D1P, D], f16, kind="ExternalInput")
    # cst: cols 0:128 identity[128,128]; cols 128+n*128 rep_sel_n in rows 0:32
    cst_d = nc.dram_tensor("cst", [P, P + N_LOC * P], f16,
                           kind="ExternalInput")
    out_d = nc.dram_tensor("out", [N_LOC, C, HW], f16, kind="ExternalOutput")

    # mm2 rhs column spans per t'-class (tp4): list of (g0, gw)
    def mm2_spans(tp4):
        if tp4 == 0:
            return [(0, 4), (4, 2), (6, 1)]
        return [(0, 4), (4, 2)]

    with tile.TileContext(nc) as tc:
        with tc.tile_pool(name="const", bufs=1) as const:
            cst = const.tile([P, P + N_LOC * P], f16, tag="cst")
            nc.scalar.dma_start(cst[:, :], cst_d.ap()[:, :])
            ident = cst[:, 0:P]

            KT = [const.tile([P, NM], f16, tag=f"KT{kc}", name=f"KT{kc}")
                  for kc in range(KC)]
            V_all = const.tile([NM, C], f16, tag="V_all")
            apad = [const.tile([P, NT * MP], f16, tag=f"apad{i}",
                               name=f"apad{i}") for i in range(2)]
            for i in range(2):
                nc.vector.memset(apad[i][:, :].bitcast(f32), 0.0)

            with tc.tile_pool(name="wtp", bufs=1) as wtp, \
                 tc.tile_pool(name="psum0", bufs=1, space=PSUM) as psum0:
                wt_all = wtp.tile([P, 7 * D], f16, tag="wt_all")
                gft_all = wtp.tile([P, 7 * NM], f16, tag="gft_all")
                # consts go on the scalar HWDGE queue so item 0's x load can
                # run concurrently on the sync queue
                nc.scalar.dma_start(
                    gft_all[:, :].rearrange("p (k e) -> p k e", e=NM),
                    gft_d.ap().rearrange("(k p) e -> p k e", p=P))
                nc.scalar.dma_start(
                    wt_all[:, :].rearrange("p (k e) -> p k e", e=D),
                    wt_d.ap().rearrange("(k p) e -> p k e", p=P))

                def wt_sb(i):
                    return wt_all[:, i * D:(i + 1) * D]

                def gft_sb(i):
                    return gft_all[:, i * NM:(i + 1) * NM]

                # K^T chunks: kvT[j] = wt[:, j*128:+128]^T-contracted with gft
                for j in range(KC):
                    kps = psum0.tile([P, NM], f32, tag=f"kps{j}",
                                     name=f"kps{j}")
                    for i in range(7):
                        nc.tensor.matmul(
                            kps[:, :], wt_sb(i)[:, j * P:(j + 1) * P],
                            gft_sb(i), start=(i == 0), stop=(i == 6))
                    nc.vector.tensor_scalar(KT[j][:, :], kps[:, :], 0.0, 6.0,
                                            op0=Alu.max, op1=Alu.min)
                # V for all items: [nm=32, c]
                vps = psum0.tile([NM, C], f32, tag="vps")
                for i in range(7):
                    nc.tensor.matmul(vps[:, :], gft_sb(i),
                                     wt_sb(i)[:, C:2 * C],
                                     start=(i == 0), stop=(i == 6))
                nc.vector.tensor_scalar(V_all[:, :], vps[:, :], 0.0, 6.0,
                                        op0=Alu.max, op1=Alu.min)

            with (
                tc.tile_pool(name="xp", bufs=3) as xp,
                tc.tile_pool(name="osb", bufs=2) as osb,
                tc.tile_pool(name="sm", bufs=4) as sm,
                tc.tile_pool(name="aTp", bufs=2) as aTp,
                tc.tile_pool(name="vrp", bufs=2) as vrp,
                tc.tile_pool(name="scp", bufs=2, space=PSUM) as scp,
                tc.tile_pool(name="tpp", bufs=2, space=PSUM) as tpp,
                tc.tile_pool(name="vpsm", bufs=1, space=PSUM) as vpsm,
                tc.tile_pool(name="pso", bufs=3, space=PSUM) as pso,
            ):
                rr = [0]

                def gen_out(n, aT, V_rep, xt):
                    # mm2 + residual + store for item n; one span per yield
                    # so it interleaves with the next item's phases.
                    # Residual alternates between a direct DVE
                    # tensor-add(psum, x) and an ACT copy + GPSIMD in-place
                    # add, spreading the psum-drain work over three engines.
                    ot = osb.tile([P, KC * XPAD], f16, tag="o", name="ot")
                    for kc in range(KC):
                        base = kc * XPAD
                        for tp4 in range(N_LOC):
                            pbase = MP * tp4
                            for (g0, gw) in mm2_spans(tp4):
                                po = pso.tile([P, 4 * P], f32, tag="po",
                                              name="po")
                                wmm = gw * P
                                nc.tensor.matmul(
                                    po[:, :wmm],
                                    V_rep[pbase:pbase + MP,
                                          kc * P:(kc + 1) * P],
                                    aT[pbase:pbase + MP,
                                       g0 * P:g0 * P + wmm],
                                    start=True, stop=True,
                                    tile_position=(pbase, 0))
                                lo = tp4 * P + g0 * 4 * P
                                via_act = rr[0] % 2 == 1
                                rr[0] += 1
                                if gw == 1:
                                    wv = P if lo + P <= HW else HW - lo
                                    dst = ot[:, base + lo:base + lo + wv]
                                    xv = xt[:, base + lo:base + lo + wv]
                                    if via_act:
                                        nc.scalar.copy(dst, po[:, :wv])
                                        nc.gpsimd.tensor_add(dst, dst, xv)
                                    else:
                                        nc.vector.tensor_add(dst, po[:, :wv],
                                                             xv)
                                else:
                                    span = ot[:, base + lo:base + lo +
                                              gw * 4 * P]
                                    dst = span.rearrange(
                                        "p (g z) -> p g z",
                                        z=4 * P)[:, :, 0:P]
                                    xv = xt[:, base + lo:base + lo +
                                            gw * 4 * P].rearrange(
                                        "p (g z) -> p g z",
                                        z=4 * P)[:, :, 0:P]
                                    po3 = po[:, :wmm].rearrange(
                                        "p (g z) -> p g z", z=P)
                                    if via_act:
                                        nc.scalar.copy(dst, po3)
                                        nc.gpsimd.tensor_add(dst, dst, xv)
                                    else:
                                        nc.vector.tensor_add(dst, po3, xv)
                                yield
                        eng = nc.gpsimd if (n + kc) % 2 == 0 else nc.scalar
                        eng.dma_start(
                            out_d.ap()[n, kc * P:(kc + 1) * P, :],
                            ot[:, base:base + HW])
                        yield

                def drain(gen, steps):
                    if gen is None:
                        return None
                    try:
                        for _ in range(steps):
                            next(gen)
                    except StopIteration:
                        return None
                    return gen

                def load_x(n):
                    xt = xp.tile([P, KC * XPAD], f16, tag="x", name="xt")
                    eng = nc.sync if n % 2 == 0 else nc.scalar
                    eng.dma_start(
                        xt[:, :].rearrange("p (k z) -> p k z",
                                           z=XPAD)[:, :, 0:HW],
                        xs_d.ap()[n].rearrange("(k p) h -> p k h", p=P))
                    return xt

                outgen = None
                xts = {0: load_x(0)}
                for n in range(N_LOC):
                    if n + 1 < N_LOC:
                        xts[n + 1] = load_x(n + 1)
                    xt = xts.pop(n)

                    def xsl(kc, lo, w):
                        return xt[:, kc * XPAD + lo:kc * XPAD + lo + w]

                    # V_n replicated at partitions 0/32/64/96
                    vp = vpsm.tile([P, C], f32, tag="vp")
                    nc.tensor.matmul(
                        vp[:, :], cst[0:NM, P + n * P:P + (n + 1) * P],
                        V_all[:, :], start=True, stop=True)
                    V_rep = vrp.tile([P, C], f16, tag="vr", name="vr")
                    nc.scalar.copy(V_rep[:, :], vp[:, :])

                    # scores [hw_p, m] accumulated over c-chunks
                    sc = scp.tile([P, NT * M], f32, tag="sc", name="sc")
                    for t in range(NT):
                        pt = P if t < NT - 1 else HW - (NT - 1) * P
                        for kc in range(KC):
                            nc.tensor.matmul(
                                sc[0:pt, t * M:(t + 1) * M],
                                xsl(kc, t * P, pt),
                                KT[kc][:, n * M:(n + 1) * M],
                                start=(kc == 0), stop=(kc == KC - 1))
                        outgen = drain(outgen, 1)

                    # softmax over m (free dim), 128-way partition parallel
                    nc.vector.memset(sc[64:P, (NT - 1) * M:NT * M], 0.0)
                    sc3 = sc[:, :].rearrange("p (t m) -> p t m", m=M)
                    nmx = sm.tile([P, NT], f32, tag="nmx")
                    nc.vector.tensor_reduce(nmx[:, :], sc3,
                                            axis=mybir.AxisListType.X,
                                            op=Alu.max, negate=True)
                    nmx_b = nmx[:, :].unsqueeze(-1).broadcast_to([P, NT, M])
                    e = sm.tile([P, NT * M], f32, tag="e")
                    e3 = e[:, :].rearrange("p (t m) -> p t m", m=M)
                    nc.vector.tensor_add(e3, sc3, nmx_b)
                    nc.scalar.activation(e[:, :], e[:, :], Act.Exp)
                    den = sm.tile([P, NT], f32, tag="den")
                    nc.vector.tensor_reduce(den[:, :], e3,
                                            axis=mybir.AxisListType.X,
                                            op=Alu.add)
                    r = sm.tile([P, NT], f32, tag="r")
                    nc.vector.reciprocal(r[:, :], den[:, :])
                    r_b = r[:, :].unsqueeze(-1).broadcast_to([P, NT, M])
                    ap_t = apad[n % 2]
                    a3 = ap_t[:, :].rearrange("p (t m) -> p t m",
                                              m=MP)[:, :, 0:M]
                    nc.vector.tensor_mul(a3, e3, r_b)
                    outgen = drain(outgen, 2)

                    # batched transposes: 4 hw-tiles per [128,128] block
                    aT = aTp.tile([P, 7 * P], f16, tag="aT", name="aT")
                    for g in range(7):
                        wg = P if g < 6 else MP
                        tp = tpp.tile([P, P], f16, tag="tp", name="tp")
                        nc.tensor.transpose(tp[0:wg, :],
                                            ap_t[:, g * P:g * P + wg],
                                            ident[:, :])
                        nc.scalar.copy(aT[0:wg, g * P:(g + 1) * P],
                                       tp[0:wg, :])
                        outgen = drain(outgen, 1)

                    # flush previous item's output phase, then queue ours
                    while outgen is not None:
                        outgen = drain(outgen, 4)
                    outgen = gen_out(n, aT, V_rep, xt)
                while outgen is not None:
                    outgen = drain(outgen, 4)

    nc.compile()
    return nc


def get_nc():
    if "nc" not in _cache:
        _cache["nc"] = _build()
    return _cache["nc"]


def make_in_maps(x, global_feature, W_kv, b_kv):
    x = np.asarray(x, np.float16).reshape(N, C, HW)
    wt = np.zeros((D1P, D), np.float16)
    wt[:D] = np.asarray(W_kv, np.float32).T.astype(np.float16)
    wt[D] = np.asarray(b_kv, np.float32).astype(np.float16)
    gf = np.asarray(global_feature, np.float32)
    cst = np.zeros((P, P + N_LOC * P), np.float16)
    cst[:, :P] = np.eye(P, dtype=np.float16)
    for n in range(N_LOC):
        for p in range(P):
            m = p % MP
            if m < M:
                cst[n * M + m, P + n * P + p] = 1.0
    in_maps = []
    for i in range(N_CORES):
        gfl = gf[i * N_LOC:(i + 1) * N_LOC].reshape(NM, D)
        gft = np.zeros((D1P, NM), np.float16)
        gft[:D] = gfl.T.astype(np.float16)
        gft[D] = 1.0
        in_maps.append({
            "xs": np.ascontiguousarray(x[i * N_LOC:(i + 1) * N_LOC]),
            "gft": gft,
            "wt": wt,
            "cst": cst,
        })
    return in_maps


def kernel(x, global_feature, W_kv, b_kv, trace=False):
    global last_results
    from concourse.bass_utils import run_bass_kernel_spmd

    nc = get_nc()
    in_maps = make_in_maps(x, global_feature, W_kv, b_kv)
    res = run_bass_kernel_spmd(nc, in_maps, core_ids=list(range(N_CORES)),
                               trace=trace)
    last_results = res
    out = np.concatenate([res.results[i]["out"][None] for i in range(N_CORES)],
                         axis=0)
    return out.reshape(N, C, H, W).astype(np.float32)
